# revision 36
# baseline (speedup 1.0000x reference)
"""Trainium2 Bass kernel for nn_Attention_47132971106602.

Gated MHA layer: proj -> (rmsnorm+rope on q,k) -> causal attention -> *sigmoid(gate)
-> out-proj.  B=4, L=2048, HID=2048, H=16 heads, DH=128.

Sharding (8 cores, one NEFF, SPMD over different data):
  core = 2*b + g  (b = batch 0..3, g = head-group 0..1 -> heads [8g, 8g+8))
Each core computes QKV+gate proj for its 8 heads on its batch, full causal
attention for those heads, and a partial out-proj (contraction over its 1024
head-dims).  The pair of cores holding one batch produce partial sums of the
final (L, HID) output; the host adds the two partials (TP unshard).

v2 changes over the first working version:
  - proj(h) instruction emission is zipped with attn(h-1) (and out-proj with
    attn(7)) so the ACT-bound attention stretches are filled with proj
    matmuls in the in-order PE queue.
  - single activation-table set (natural_log_exp_and_others): rmsnorm rsqrt
    is computed as exp(-0.5*ln(x)); the sigmoid gate as 1/(1+exp(-g)) with
    the reciprocal on the DVE.  Kills the 2-per-head ACT_TABLE_LOADs.
  - all weight/activation DRAM tensors are host-pre-tiled so every DMA moves
    contiguous multi-KB per-partition segments.
  - PSUM: pss 3 bufs (scores run further ahead of exp), pav 2.
  - y / wout DMAs alternate across both hwdge queues.
"""

import os
import sys

sys.path.insert(0, "/opt/trn_rl_repo")

import math
import numpy as np

import concourse.bass as bass
import concourse.bass_isa as bass_isa
import concourse.tile as tile
from concourse import bacc
from concourse import mybir
from concourse.bass_utils import run_bass_kernel_spmd

F16 = mybir.dt.float16
F32 = mybir.dt.float32

HID = 2048
H = 16
DH = 128
L = 2048
B = 4
HPC = 8            # heads per core
KC = HID // 128    # 16 contraction chunks
TQ = 512           # token chunk (matmul free dim)
NTQ = L // TQ      # 4
NTK = L // 128     # 16 token/key tiles
THETA = 10000.0
EPS = 1e-5
SCALE = 1.0 / math.sqrt(DH)
EXP_BIAS = -2.7725887222397811  # -4*ln2; cancels in softmax normalization

_NC_CACHE = {}

_ACT_JSON_SRC = (
    "/nix/store/z022hj2nvbm3nwdizlisq4ylc0y7rd6q-python3-3.13.14-env/lib/"
    "python3.13/site-packages/neuronxcc/pwp/pwp_bin_trainium/act_info.json"
)


def _install_act_tables():
    """Reorder act table sets so ln/exp/copy all resolve to one set -- the
    kernel then needs a single ACT_TABLE_LOAD instead of one per swap."""
    import json
    try:
        with open(_ACT_JSON_SRC) as f:
            d = json.load(f)
        src_dir = os.path.dirname(_ACT_JSON_SRC)
    except OSError:
        import glob as _g
        cands = _g.glob(
            "/nix/store/*python*env/lib/python3*/site-packages/neuronxcc/pwp/"
            "pwp_bin_trainium/act_info.json"
        )
        if not cands:
            return
        with open(cands[0]) as f:
            d = json.load(f)
        src_dir = os.path.dirname(cands[0])
    sets = d.get("act_func_sets", [])
    sets.sort(key=lambda s0: 0 if s0.get("name") == "natural_log_exp_and_others" else 1)
    outdir = "/tmp/kernel_act_tables"
    os.makedirs(outdir, exist_ok=True)
    out = os.path.join(outdir, "act_info.json")
    with open(out, "w") as f:
        json.dump(d, f)
    for fn in os.listdir(src_dir):
        dst = os.path.join(outdir, fn)
        if fn != "act_info.json" and not os.path.exists(dst):
            import shutil as _sh
            _sh.copyfile(os.path.join(src_dir, fn), dst)
    os.environ["BASS_ACT_ROOT_JSON_PATH"] = out

    # the compile-time set selection (insert_act_table_loads) reads the
    # default act_info.json via findActInfoFile -- point it at ours so the
    # chosen sets match the runtime tables and ln/exp/copy share one set.
    import concourse.hw_specs as hw_specs

    def _tables(module_arch):
        return {
            ent["name"]: {
                mybir.ActivationFunctionType.from_pwp(v)
                for v in ent["act"].keys()
            }
            for ent in d["act_func_sets"]
        }

    hw_specs.get_activation_tables = _tables
    bacc.get_activation_tables = _tables
    try:
        import concourse.bass_interp as bass_interp
        bass_interp.get_activation_tables = _tables
    except ImportError:
        pass


A = mybir.ActivationFunctionType


def build_nc():
    nc = bacc.Bacc("TRN2", target_bir_lowering=False, debug=False)

    # pre-tiled layouts: partition dim first, per-partition contiguous chunks
    xT = nc.dram_tensor("xT", [128, NTK, KC, 128], F16, kind="ExternalInput")
    wq = nc.dram_tensor("wq", [128, HPC, KC, DH], F16, kind="ExternalInput")
    wk = nc.dram_tensor("wk", [128, HPC, KC, DH], F16, kind="ExternalInput")
    wv = nc.dram_tensor("wv", [128, 2, KC, TQ], F16, kind="ExternalInput")
    wg = nc.dram_tensor("wg", [128, HPC, KC, DH], F16, kind="ExternalInput")
    wout = nc.dram_tensor("wout", [128, HPC, HID], F16, kind="ExternalInput")
    cosT = nc.dram_tensor("cosT", [DH, L], F16, kind="ExternalInput")
    sinT = nc.dram_tensor("sinT", [DH, L], F16, kind="ExternalInput")
    masks = nc.dram_tensor("masks", [128, TQ], F16, kind="ExternalInput")
    y = nc.dram_tensor("y", [L, HID], F16, kind="ExternalOutput")
    rk_dram = nc.dram_tensor("rk_scratch", [HPC, L], F32)

    with tile.TileContext(nc) as tc:
        with tc.tile_pool(name="big", bufs=1) as big:
            V_sb = big.tile([128, NTK, HPC * DH], F16, tag="V")       # token-major
            gated = big.tile([128, HPC, L], F16, tag="gated")         # col-major
            cos_sb = big.tile([128, L], F16, tag="cos")
            sin_sb = big.tile([128, L], F16, tag="sin")
            mask_sb = big.tile([128, TQ], F16, tag="mask")
            ones_sb = big.tile([128, 1], F16, tag="ones")
            epsb_sb = big.tile([128, 1], F32, tag="epsb")
            eps128_sb = big.tile([128, 1], F32, tag="eps128")
            ebias_sb = big.tile([128, 1], F32, tag="ebias")

            nc.vector.memset(ones_sb, 1.0)
            nc.vector.memset(epsb_sb, EPS)
            nc.vector.memset(eps128_sb, DH * EPS)
            nc.vector.memset(ebias_sb, EXP_BIAS)

            # attention-lifetime pools opened before xtp so releases stack
            rp_ctx = tc.tile_pool(name="rp", bufs=2)
            rp = rp_ctx.__enter__()
            gp_ctx = tc.tile_pool(name="gp", bufs=2)
            gp = gp_ctx.__enter__()
            sp_ctx = tc.tile_pool(name="sp", bufs=2)
            sp = sp_ctx.__enter__()
            gw_ctx = tc.tile_pool(name="gw", bufs=1)
            gwp = gw_ctx.__enter__()
            ep_ctx = tc.tile_pool(name="ep", bufs=4)
            epool = ep_ctx.__enter__()
            pss_ctx = tc.tile_pool(name="pss", bufs=3, space="PSUM")
            pss = pss_ctx.__enter__()
            pav_ctx = tc.tile_pool(name="pav", bufs=2, space="PSUM")
            pav = pav_ctx.__enter__()
            pv1_ctx = tc.tile_pool(name="pv1", bufs=1, space="PSUM")
            pv1 = pv1_ctx.__enter__()

            xtp = tc.tile_pool(name="xtp", bufs=1)
            xtp_pool = xtp.__enter__()
            xT_sb = xtp_pool.tile([128, NTK, KC, 128], F16, tag="xT")

            # ---------------- V projection (all heads, token-major) -------------
            with (
                tc.tile_pool(name="wvp", bufs=1) as wvp,
                tc.tile_pool(name="pv", bufs=2, space="PSUM") as pvp,
            ):
                # vc=1 weights live in the gated tile's space (gated is only
                # written during attention, long after these reads finish)
                wv1_sb = gated[:, 0:4, :].rearrange("p h (a t) -> p (h a) t", a=4)
                dma_engs = [nc.sync, nc.scalar, nc.gpsimd]

                # quarter DMAs rotating over 3 queues so chains start early
                def wv_dma(wv_sb, vc):
                    for q4 in range(4):
                        eng = dma_engs[q4 % 3]
                        ks = slice(4 * q4, 4 * q4 + 4)
                        eng.dma_start(out=wv_sb[:, ks, :], in_=wv[:, vc, ks, :])

                wv0_sb = wvp.tile([128, KC, TQ], F16, tag="wv", name="wv0")
                for q8 in range(8):
                    eng = dma_engs[q8 % 3]
                    ks = slice(2 * q8, 2 * q8 + 2)
                    eng.dma_start(out=wv0_sb[:, ks, :], in_=wv[:, 0, ks, :])
                for tch in range(NTK):
                    eng = dma_engs[(tch + 2) % 3]
                    eng.dma_start(out=xT_sb[:, tch, :, :], in_=xT[:, tch, :, :])
                wv_dma(wv1_sb, 1)
                nc.gpsimd.dma_start(out=cos_sb, in_=cosT[:, :])
                nc.gpsimd.dma_start(out=sin_sb, in_=sinT[:, :])
                nc.gpsimd.dma_start(out=mask_sb, in_=masks[:, :])
                def v_chain(ps, wv_sb, t, k0, k1):
                    for kc in range(k0, k1):
                        nc.tensor.matmul(
                            ps,
                            xT_sb[:, t, kc, :],
                            wv_sb[:, kc, :],
                            start=(kc == 0),
                            stop=(kc == KC - 1),
                        )

                for vc in range(2):
                    wv_sb = wv0_sb if vc == 0 else wv1_sb
                    if vc == 0:
                        # first two chains split in half so the PE starts on
                        # the first half of wv0 while the rest streams in
                        ps0 = pvp.tile([128, TQ], F32, tag="pv", name="pv_t0")
                        ps1 = pvp.tile([128, TQ], F32, tag="pv", name="pv_t1")
                        v_chain(ps0, wv_sb, 0, 0, 8)
                        v_chain(ps1, wv_sb, 1, 0, 8)
                        v_chain(ps0, wv_sb, 0, 8, KC)
                        nc.vector.tensor_copy(V_sb[:, 0, 0:TQ], ps0)
                        v_chain(ps1, wv_sb, 1, 8, KC)
                        nc.vector.tensor_copy(V_sb[:, 1, 0:TQ], ps1)
                        trange = range(2, NTK)
                    else:
                        trange = range(NTK)
                    for t in trange:
                        ps = pvp.tile([128, TQ], F32, tag="pv")
                        v_chain(ps, wv_sb, t, 0, KC)
                        nc.vector.tensor_copy(
                            V_sb[:, t, vc * TQ:(vc + 1) * TQ], ps)

            # ---------------- per-head proj + attention -------------------------
            if True:
                wp_ctx = tc.tile_pool(name="wp", bufs=2)
                wp = wp_ctx.__enter__()
                wgp_ctx = tc.tile_pool(name="wgp", bufs=1)
                wgp = wgp_ctx.__enter__()
                qkp_ctx = tc.tile_pool(name="qkp", bufs=2)
                qkp = qkp_ctx.__enter__()
                pp_ctx = tc.tile_pool(name="pp", bufs=2, space="PSUM")
                pp = pp_ctx.__enter__()

                def load_qk(h):
                    wq_sb = wp.tile([128, KC, DH], F16, tag="wq", name=f"wq{h}")
                    wk_sb = wp.tile([128, KC, DH], F16, tag="wk", name=f"wk{h}")
                    nc.scalar.dma_start(out=wq_sb[:, 0:8, :], in_=wq[:, h, 0:8, :])
                    nc.sync.dma_start(out=wq_sb[:, 8:16, :], in_=wq[:, h, 8:16, :])
                    nc.scalar.dma_start(out=wk_sb[:, 0:8, :], in_=wk[:, h, 0:8, :])
                    nc.sync.dma_start(out=wk_sb[:, 8:16, :], in_=wk[:, h, 8:16, :])
                    return wq_sb, wk_sb

                def proj_head(h, out, qk, nextqk):
                    wq_sb, wk_sb = qk
                    # prefetch next head's q/k weights one head early
                    if h + 1 < HPC:
                        nextqk[h + 1] = load_qk(h + 1)
                    wg_sb = wgp.tile([128, KC, DH], F16, tag="wg", name=f"wg{h}")
                    nc.scalar.dma_start(out=wg_sb[:, 0:8, :], in_=wg[:, h, 0:8, :])
                    nc.sync.dma_start(out=wg_sb[:, 8:16, :], in_=wg[:, h, 8:16, :])
                    roped = {}
                    rkcol = rp.tile([128, NTK], F32, tag="rkcol", name=f"rkcol{h}")

                    # rms tail for a chunk, delayed one chunk so the PE never
                    # waits on the DVE-produced sq
                    def emit_rms(tname, c, sq, t1, rt):
                        cs = slice(c * TQ, (c + 1) * TQ)
                        ms = pv1.tile([1, TQ], F32, tag="v1")
                        nc.tensor.matmul(ms, ones_sb, sq, start=True, stop=True)
                        # rsqrt via exp(-0.5*ln(.)) -- keeps one act table
                        # set; rows 32/0 of one tile avoid in-place act
                        rn2 = sp.tile([33, TQ], F32, tag="rn2")
                        if tname == "q":
                            # rn_q = 1/sqrt(mean(q^2)+eps)
                            nc.scalar.activation(
                                rn2[32:33, :], ms, A.Ln,
                                bias=epsb_sb[:1, :], scale=1.0 / DH)
                        else:
                            # rn_k' = 1/sqrt(DH*(mean(k^2)+eps)) = rn_k/sqrt(DH)
                            nc.scalar.activation(
                                rn2[32:33, :], ms, A.Ln,
                                bias=eps128_sb[:1, :], scale=1.0)
                        nc.scalar.activation(
                            rn2[0:1, :], rn2[32:33, :], A.Exp, scale=-0.5)
                        if tname == "q":
                            rnb = sp.tile([128, TQ], F32, tag="arr")
                            nc.gpsimd.partition_broadcast(rnb, rn2[0:1, :])
                            nc.vector.tensor_mul(rt[:, cs], t1, rnb)
                        else:
                            # k-side rms scale rides the softmax-exp scale AP
                            nc.sync.dma_start(
                                out=rk_dram[h, cs], in_=rn2[0:1, :])
                            nc.sync.dma_start(
                                out=rkcol[:, c * 4:(c + 1) * 4],
                                in_=rk_dram[h, cs].rearrange(
                                    "(r j) -> j r", j=128),
                            )

                    pend = None
                    for tname, w_sb in (("q", wq_sb), ("k", wk_sb)):
                        rt = rp.tile([128, L], F16, tag=f"{tname}r", name=f"{tname}r{h}")
                        for c in range(NTQ):
                            cs = slice(c * TQ, (c + 1) * TQ)
                            ps = pp.tile([128, TQ], F32, tag="mm")
                            for kc in range(KC):
                                nc.tensor.matmul(
                                    ps,
                                    w_sb[:, kc, :],
                                    xT_sb[:, 4 * c:4 * c + 4, kc, :],
                                    start=(kc == 0),
                                    stop=(kc == KC - 1),
                                )
                            qt = qkp.tile([128, TQ], F16, tag="qt")
                            nc.vector.tensor_copy(qt, ps)
                            qsw = qkp.tile([128, TQ], F16, tag="qsw")
                            nc.sync.dma_start(out=qsw[0:64, :], in_=qt[64:128, :])
                            nc.sync.dma_start(out=qsw[64:128, :], in_=qt[0:64, :])
                            sq = sp.tile([128, TQ], F16, tag="sq")
                            nc.vector.tensor_mul(sq, qt, qt)
                            # rope halves: t1 = qt*cos ; qsw *= sin_signed
                            t1 = sp.tile([128, TQ], F16, tag="t1")
                            nc.vector.tensor_mul(t1, qt, cos_sb[:, cs])
                            nc.vector.tensor_mul(qsw, qsw, sin_sb[:, cs])
                            if tname == "q":
                                nc.vector.tensor_add(t1, t1, qsw)
                            else:
                                # k rope lands in rt; rms scale applied at exp
                                nc.vector.tensor_add(rt[:, cs], t1, qsw)
                            if pend is not None:
                                emit_rms(*pend)
                            pend = (tname, c, sq, t1, rt)
                            yield
                        roped[tname] = rt
                    # gate: store exp(-g); sigmoid finished in the attn epilogue
                    gt = gp.tile([128, L], F16, tag="gt", name=f"g{h}")
                    for c in range(NTQ):
                        cs = slice(c * TQ, (c + 1) * TQ)
                        ps = pp.tile([128, TQ], F32, tag="mm")
                        for kc in range(KC):
                            nc.tensor.matmul(
                                ps,
                                wg_sb[:, kc, :],
                                xT_sb[:, 4 * c:4 * c + 4, kc, :],
                                start=(kc == 0),
                                stop=(kc == KC - 1),
                            )
                        nc.scalar.activation(gt[:, cs], ps, A.Exp, scale=-1.0)
                        if pend is not None:
                            emit_rms(*pend)
                            pend = None
                        yield
                    out.update(q=roped["q"], k=roped["k"], gt=gt, rkcol=rkcol)

                def attn_head(h, qr, kr, gt, rkcol, corder=None):
                    for c in (corder if corder is not None else range(NTQ)):
                        cs = slice(c * TQ, (c + 1) * TQ)
                        nkt = 4 * c + 4
                        pavt = pav.tile([128, TQ], F32, tag="av")
                        esum = sp.tile([128, TQ], F16, tag="esum", name=f"es{h}_{c}")
                        pend = []
                        for kt in range(nkt):
                            r = kt - 4 * c
                            co = max(0, 128 * r)   # masked-out column prefix
                            ncs = slice(c * TQ + co, (c + 1) * TQ)
                            pst = pss.tile([128, TQ], F32, tag="s")
                            nc.tensor.matmul(
                                pst[:, co:],
                                kr[:, kt * 128:(kt + 1) * 128],
                                qr[:, ncs],
                                start=True,
                                stop=True,
                            )
                            e = epool.tile([128, TQ], F16, tag="e")
                            nc.scalar.activation(
                                e[:, co:], pst[:, co:], A.Exp,
                                scale=rkcol[:, kt:kt + 1], bias=ebias_sb[:, :],
                            )
                            if r >= 0:
                                nc.vector.tensor_mul(
                                    e[:, co:], e[:, co:], mask_sb[:, 0:TQ - co])
                            if kt == 0:
                                nc.vector.tensor_copy(esum, e)
                            else:
                                nc.vector.tensor_add(
                                    esum[:, co:], esum[:, co:], e[:, co:])
                            pend.append((kt, e, co))
                            if len(pend) > 2:
                                pkt, pe, pco = pend.pop(0)
                                nc.tensor.matmul(
                                    pavt[:, pco:],
                                    V_sb[:, pkt, h * DH:(h + 1) * DH],
                                    pe[:, pco:],
                                    start=(pkt == 0), stop=False,
                                )
                            if kt % 2 == 1:
                                yield
                        while pend:
                            pkt, pe, pco = pend.pop(0)
                            nc.tensor.matmul(
                                pavt[:, pco:], V_sb[:, pkt, h * DH:(h + 1) * DH],
                                pe[:, pco:],
                                start=(pkt == 0), stop=(len(pend) == 0),
                            )
                        den = pv1.tile([1, TQ], F32, tag="v1", name=f"dn{h}_{c}")
                        nc.tensor.matmul(den, ones_sb, esum, start=True, stop=True)
                        rd = sp.tile([33, TQ], F32, tag="rn2", name=f"rd{h}_{c}")
                        nc.vector.reciprocal_approx_fast(out=rd[0:1, :], in_=den)
                        rdb = sp.tile([128, TQ], F32, tag="arr", name=f"rdb{h}_{c}")
                        nc.gpsimd.partition_broadcast(rdb, rd[0:1, :])
                        # finish the sigmoid: gate = 1/(1+exp(-g))
                        gden = sp.tile([128, TQ], F32, tag="arr", name=f"gd{h}_{c}")
                        nc.vector.tensor_scalar_add(gden, gt[:, cs], 1.0)
                        grec = gwp.tile([128, TQ], F32, tag="grec", name=f"gr{h}_{c}")
                        nc.vector.reciprocal_approx_fast(out=grec, in_=gden)
                        tn = sp.tile([128, TQ], F16, tag="tn")
                        # tn = av / den
                        nc.vector.scalar_tensor_tensor(
                            out=tn, in0=pavt, scalar=1.0, in1=rdb,
                            op0=mybir.AluOpType.mult, op1=mybir.AluOpType.mult)
                        nc.vector.tensor_mul(gated[:, h, cs], tn, grec)
                        yield ("c", c)

                def zip2(pg, ag, ratio=3):
                    done_p = done_a = False
                    while not (done_p and done_a):
                        if not done_p:
                            try:
                                next(pg)
                            except StopIteration:
                                done_p = True
                        for _ in range(ratio):
                            if done_a:
                                break
                            try:
                                next(ag)
                            except StopIteration:
                                done_a = True

                ag = None
                outs = {}
                nextqk = {0: load_qk(0)}
                for h in range(HPC):
                    out = {}
                    pg = proj_head(h, out, nextqk.pop(h), nextqk)
                    if ag is None:
                        for _ in pg:
                            pass
                    else:
                        zip2(pg, ag)
                    outs[h] = out
                    # the last head's attention runs longest-chunk-first so
                    # the final out-proj rows only wait on the short chunk
                    corder = [3, 2, 1, 0] if h == HPC - 1 else None
                    ag = attn_head(h, out["q"], out["k"], out["gt"],
                                   out["rkcol"], corder)

                # free proj-phase pools; xT no longer needed after proj7
                pp_ctx.__exit__(None, None, None)
                qkp_ctx.__exit__(None, None, None)
                wgp_ctx.__exit__(None, None, None)
                wp_ctx.__exit__(None, None, None)
                xtp.__exit__(None, None, None)

                # ---------------- out projection, zipped with attn(7) ------------
                with (
                    tc.tile_pool(name="wo", bufs=1) as wo,
                    tc.tile_pool(name="yp", bufs=2) as yp,
                    tc.tile_pool(name="py", bufs=2, space="PSUM") as pyp,
                ):
                    wout_sb = wo.tile([128, HPC, HID], F16, tag="wout")
                    for hc in range(HPC):
                        eng = dma_engs[hc % 3]
                        eng.dma_start(out=wout_sb[:, hc, :], in_=wout[:, hc, :])

                    def outproj_chunk(trange):
                        for t in trange:
                            ysb = yp.tile([128, HID], F16, tag="y")
                            for oc in range(NTQ):
                                ocs = slice(oc * TQ, (oc + 1) * TQ)
                                ps = pyp.tile([128, TQ], F32, tag="ym")
                                for hc in range(HPC):
                                    nc.tensor.matmul(
                                        ps,
                                        gated[:, hc, t * 128:(t + 1) * 128],
                                        wout_sb[:, hc, ocs],
                                        start=(hc == 0),
                                        stop=(hc == HPC - 1),
                                    )
                                if oc % 2 == 0:
                                    nc.vector.tensor_copy(ysb[:, ocs], ps)
                                else:
                                    nc.scalar.copy(ysb[:, ocs], ps)
                                # stream each quarter out as soon as it is copied
                                eng = nc.sync if oc % 2 == 0 else nc.scalar
                                eng.dma_start(
                                    out=y[t * 128:(t + 1) * 128, ocs],
                                    in_=ysb[:, ocs])

                    # interleave: after attn(7) finishes chunk c, emit the
                    # out-proj rows that depend on it (t = 4c .. 4c+3)
                    for step in ag:
                        if isinstance(step, tuple) and step[0] == "c":
                            c = step[1]
                            outproj_chunk(range(4 * c, 4 * c + 4))

                for ctx in (pv1_ctx, pav_ctx, pss_ctx, ep_ctx, gw_ctx,
                            sp_ctx, gp_ctx, rp_ctx):
                    ctx.__exit__(None, None, None)

    nc.compile()
    return nc


def _host_tables():
    half = DH // 2
    inv_freq = 1.0 / (THETA ** (np.arange(half, dtype=np.float64) * 2.0 / DH))
    pos = np.arange(L, dtype=np.float64)
    ang = pos[:, None] * inv_freq[None, :]          # (L, 64)
    cos = np.cos(ang).T                             # (64, L)
    sin = np.sin(ang).T
    cosT = np.concatenate([cos, cos], axis=0).astype(np.float16)        # (128, L)
    sinT = np.concatenate([-sin, sin], axis=0).astype(np.float16)
    j = np.arange(128)[:, None]
    i = np.arange(TQ)[None, :]
    mask = (j <= i).astype(np.float16)                                  # (128,512)
    return cosT, sinT, mask


def _tile_x(xb):
    # (HID, L) -> [k, tch, kc, tin] = [128, NTK, KC, 128]
    return np.ascontiguousarray(
        xb.reshape(KC, 128, NTK, 128).transpose(1, 2, 0, 3)).astype(np.float16)


def _tile_w(w, inner, last):
    # (HID, inner*last) -> [k, inner, kc, last]
    return np.ascontiguousarray(
        w.reshape(KC, 128, inner, last).transpose(1, 2, 0, 3)).astype(np.float16)


def _run(hidden_states, W_qkvg, W_out, trace=False, trace_cores=None):
    key = "nc"
    if key not in _NC_CACHE:
        _install_act_tables()
        _NC_CACHE[key] = build_nc()
    nc = _NC_CACHE[key]

    hidden_states = np.asarray(hidden_states)
    W_qkvg = np.asarray(W_qkvg)
    W_out = np.asarray(W_out)

    cosT, sinT, mask = _host_tables()
    QKV = 3 * H * DH

    in_maps = []
    for core in range(8):
        b, g = divmod(core, 2)
        cols = slice(g * HPC * DH, (g + 1) * HPC * DH)
        wv_cols = W_qkvg[:, 2 * H * DH:3 * H * DH][:, cols]
        in_maps.append({
            "xT": _tile_x(np.ascontiguousarray(hidden_states[b].T)),
            "wq": _tile_w(W_qkvg[:, 0 * H * DH:1 * H * DH][:, cols], HPC, DH),
            "wk": _tile_w(W_qkvg[:, 1 * H * DH:2 * H * DH][:, cols], HPC, DH),
            "wv": _tile_w(wv_cols, 2, TQ),
            "wg": _tile_w(W_qkvg[:, QKV:][:, cols], HPC, DH),
            "wout": np.ascontiguousarray(
                W_out[cols, :].reshape(HPC, 128, HID).transpose(1, 0, 2)
            ).astype(np.float16),
            "cosT": cosT,
            "sinT": sinT,
            "masks": mask,
        })

    kw = {}
    if trace:
        kw["trace"] = True
        if trace_cores is not None:
            kw["trace_cores"] = trace_cores
    res = run_bass_kernel_spmd(nc, in_maps, core_ids=list(range(8)), **kw)

    out = np.empty((B, L, HID), dtype=np.float32)
    for b in range(B):
        out[b] = (res.results[2 * b]["y"].astype(np.float32)
                  + res.results[2 * b + 1]["y"].astype(np.float32))
    return out, res


def kernel(hidden_states, W_qkvg, W_out):
    trace = os.environ.get("KERNEL_TRACE", "0") == "1"
    out, res = _run(hidden_states, W_qkvg, W_out, trace=trace)
    kernel.last_results = res
    return out


if __name__ == "__main__":
    rng = np.random.default_rng(0)
    hs = rng.standard_normal((B, L, HID), dtype=np.float32)
    wqkvg = (rng.standard_normal((HID, QKV_ := 3 * H * DH + HID), dtype=np.float32) * 0.02)
    wout = (rng.standard_normal((HID, HID), dtype=np.float32) * 0.02)
    out = kernel(hs, wqkvg, wout)
    print(out.shape, out.dtype)


# revision 43
# speedup vs baseline: 1.0015x; 1.0015x over previous
"""Trainium2 Bass kernel for nn_Attention_47132971106602.

Gated MHA layer: proj -> (rmsnorm+rope on q,k) -> causal attention -> *sigmoid(gate)
-> out-proj.  B=4, L=2048, HID=2048, H=16 heads, DH=128.

Sharding (8 cores, one NEFF, SPMD over different data):
  core = 2*b + g  (b = batch 0..3, g = head-group 0..1 -> heads [8g, 8g+8))
Each core computes QKV+gate proj for its 8 heads on its batch, full causal
attention for those heads, and a partial out-proj (contraction over its 1024
head-dims).  The pair of cores holding one batch produce partial sums of the
final (L, HID) output; the host adds the two partials (TP unshard).

v2 changes over the first working version:
  - proj(h) instruction emission is zipped with attn(h-1) (and out-proj with
    attn(7)) so the ACT-bound attention stretches are filled with proj
    matmuls in the in-order PE queue.
  - single activation-table set (natural_log_exp_and_others): rmsnorm rsqrt
    is computed as exp(-0.5*ln(x)); the sigmoid gate as 1/(1+exp(-g)) with
    the reciprocal on the DVE.  Kills the 2-per-head ACT_TABLE_LOADs.
  - all weight/activation DRAM tensors are host-pre-tiled so every DMA moves
    contiguous multi-KB per-partition segments.
  - PSUM: pss 3 bufs (scores run further ahead of exp), pav 2.
  - y / wout DMAs alternate across both hwdge queues.
"""

import os
import sys

sys.path.insert(0, "/opt/trn_rl_repo")

import math
import numpy as np

import concourse.bass as bass
import concourse.bass_isa as bass_isa
import concourse.tile as tile
from concourse import bacc
from concourse import mybir
from concourse.bass_utils import run_bass_kernel_spmd

F16 = mybir.dt.float16
F32 = mybir.dt.float32

HID = 2048
H = 16
DH = 128
L = 2048
B = 4
HPC = 8            # heads per core
KC = HID // 128    # 16 contraction chunks
TQ = 512           # token chunk (matmul free dim)
NTQ = L // TQ      # 4
NTK = L // 128     # 16 token/key tiles
THETA = 10000.0
EPS = 1e-5
SCALE = 1.0 / math.sqrt(DH)
EXP_BIAS = -2.7725887222397811  # -4*ln2; cancels in softmax normalization

_NC_CACHE = {}

_ACT_JSON_SRC = (
    "/nix/store/z022hj2nvbm3nwdizlisq4ylc0y7rd6q-python3-3.13.14-env/lib/"
    "python3.13/site-packages/neuronxcc/pwp/pwp_bin_trainium/act_info.json"
)


def _install_act_tables():
    """Reorder act table sets so ln/exp/copy all resolve to one set -- the
    kernel then needs a single ACT_TABLE_LOAD instead of one per swap."""
    import json
    try:
        with open(_ACT_JSON_SRC) as f:
            d = json.load(f)
        src_dir = os.path.dirname(_ACT_JSON_SRC)
    except OSError:
        import glob as _g
        cands = _g.glob(
            "/nix/store/*python*env/lib/python3*/site-packages/neuronxcc/pwp/"
            "pwp_bin_trainium/act_info.json"
        )
        if not cands:
            return
        with open(cands[0]) as f:
            d = json.load(f)
        src_dir = os.path.dirname(cands[0])
    sets = d.get("act_func_sets", [])
    sets.sort(key=lambda s0: 0 if s0.get("name") == "natural_log_exp_and_others" else 1)
    outdir = "/tmp/kernel_act_tables"
    os.makedirs(outdir, exist_ok=True)
    out = os.path.join(outdir, "act_info.json")
    with open(out, "w") as f:
        json.dump(d, f)
    for fn in os.listdir(src_dir):
        dst = os.path.join(outdir, fn)
        if fn != "act_info.json" and not os.path.exists(dst):
            import shutil as _sh
            _sh.copyfile(os.path.join(src_dir, fn), dst)
    os.environ["BASS_ACT_ROOT_JSON_PATH"] = out

    # the compile-time set selection (insert_act_table_loads) reads the
    # default act_info.json via findActInfoFile -- point it at ours so the
    # chosen sets match the runtime tables and ln/exp/copy share one set.
    import concourse.hw_specs as hw_specs

    def _tables(module_arch):
        return {
            ent["name"]: {
                mybir.ActivationFunctionType.from_pwp(v)
                for v in ent["act"].keys()
            }
            for ent in d["act_func_sets"]
        }

    hw_specs.get_activation_tables = _tables
    bacc.get_activation_tables = _tables
    try:
        import concourse.bass_interp as bass_interp
        bass_interp.get_activation_tables = _tables
    except ImportError:
        pass


A = mybir.ActivationFunctionType


def build_nc():
    nc = bacc.Bacc("TRN2", target_bir_lowering=False, debug=False)

    # pre-tiled layouts: partition dim first, per-partition contiguous chunks
    xT = nc.dram_tensor("xT", [128, NTK, KC, 128], F16, kind="ExternalInput")
    wq = nc.dram_tensor("wq", [128, HPC, KC, DH], F16, kind="ExternalInput")
    wk = nc.dram_tensor("wk", [128, HPC, KC, DH], F16, kind="ExternalInput")
    wv = nc.dram_tensor("wv", [128, 2, KC, TQ], F16, kind="ExternalInput")
    wg = nc.dram_tensor("wg", [128, HPC, KC, DH], F16, kind="ExternalInput")
    wout = nc.dram_tensor("wout", [128, HPC, HID], F16, kind="ExternalInput")
    cosT = nc.dram_tensor("cosT", [DH, L], F16, kind="ExternalInput")
    sinT = nc.dram_tensor("sinT", [DH, L], F16, kind="ExternalInput")
    masks = nc.dram_tensor("masks", [128, TQ], F16, kind="ExternalInput")
    y = nc.dram_tensor("y", [L, HID], F16, kind="ExternalOutput")
    rk_dram = nc.dram_tensor("rk_scratch", [HPC, L], F32)

    with tile.TileContext(nc) as tc:
        with tc.tile_pool(name="big", bufs=1) as big:
            V_sb = big.tile([128, NTK, HPC * DH], F16, tag="V")       # token-major
            gated = big.tile([128, HPC, L], F16, tag="gated")         # col-major
            cos_sb = big.tile([128, L], F16, tag="cos")
            sin_sb = big.tile([128, L], F16, tag="sin")
            mask_sb = big.tile([128, TQ], F16, tag="mask")
            ones_sb = big.tile([128, 1], F16, tag="ones")
            epsb_sb = big.tile([128, 1], F32, tag="epsb")
            eps128_sb = big.tile([128, 1], F32, tag="eps128")
            ebias_sb = big.tile([128, 1], F32, tag="ebias")

            nc.vector.memset(ones_sb, 1.0)
            nc.vector.memset(epsb_sb, EPS)
            nc.vector.memset(eps128_sb, DH * EPS)
            nc.vector.memset(ebias_sb, EXP_BIAS)

            # attention-lifetime pools opened before xtp so releases stack
            rp_ctx = tc.tile_pool(name="rp", bufs=2)
            rp = rp_ctx.__enter__()
            gp_ctx = tc.tile_pool(name="gp", bufs=2)
            gp = gp_ctx.__enter__()
            sp_ctx = tc.tile_pool(name="sp", bufs=2)
            sp = sp_ctx.__enter__()
            gw_ctx = tc.tile_pool(name="gw", bufs=1)
            gwp = gw_ctx.__enter__()
            ep_ctx = tc.tile_pool(name="ep", bufs=4)
            epool = ep_ctx.__enter__()
            pss_ctx = tc.tile_pool(name="pss", bufs=3, space="PSUM")
            pss = pss_ctx.__enter__()
            pav_ctx = tc.tile_pool(name="pav", bufs=3, space="PSUM")
            pav = pav_ctx.__enter__()

            xtp = tc.tile_pool(name="xtp", bufs=1)
            xtp_pool = xtp.__enter__()
            xT_sb = xtp_pool.tile([128, NTK, KC, 128], F16, tag="xT")

            # ---------------- V projection (all heads, token-major) -------------
            with (
                tc.tile_pool(name="wvp", bufs=1) as wvp,
                tc.tile_pool(name="pv", bufs=2, space="PSUM") as pvp,
            ):
                # vc=1 weights live in the gated tile's space (gated is only
                # written during attention, long after these reads finish)
                wv1_sb = gated[:, 0:4, :].rearrange("p h (a t) -> p (h a) t", a=4)
                dma_engs = [nc.sync, nc.scalar, nc.gpsimd]

                # quarter DMAs rotating over 3 queues so chains start early
                def wv_dma(wv_sb, vc):
                    for q4 in range(4):
                        eng = dma_engs[q4 % 3]
                        ks = slice(4 * q4, 4 * q4 + 4)
                        eng.dma_start(out=wv_sb[:, ks, :], in_=wv[:, vc, ks, :])

                wv0_sb = wvp.tile([128, KC, TQ], F16, tag="wv", name="wv0")
                for q8 in range(8):
                    eng = dma_engs[q8 % 3]
                    ks = slice(2 * q8, 2 * q8 + 2)
                    eng.dma_start(out=wv0_sb[:, ks, :], in_=wv[:, 0, ks, :])
                for tch in range(NTK):
                    eng = dma_engs[(tch + 2) % 3]
                    eng.dma_start(out=xT_sb[:, tch, :, :], in_=xT[:, tch, :, :])
                wv_dma(wv1_sb, 1)
                nc.gpsimd.dma_start(out=cos_sb, in_=cosT[:, :])
                nc.gpsimd.dma_start(out=sin_sb, in_=sinT[:, :])
                nc.gpsimd.dma_start(out=mask_sb, in_=masks[:, :])
                def v_chain(ps, wv_sb, t, k0, k1):
                    for kc in range(k0, k1):
                        nc.tensor.matmul(
                            ps,
                            xT_sb[:, t, kc, :],
                            wv_sb[:, kc, :],
                            start=(kc == 0),
                            stop=(kc == KC - 1),
                        )

                for vc in range(2):
                    wv_sb = wv0_sb if vc == 0 else wv1_sb
                    if vc == 0:
                        # first two chains split in half so the PE starts on
                        # the first half of wv0 while the rest streams in
                        ps0 = pvp.tile([128, TQ], F32, tag="pv", name="pv_t0")
                        ps1 = pvp.tile([128, TQ], F32, tag="pv", name="pv_t1")
                        v_chain(ps0, wv_sb, 0, 0, 8)
                        v_chain(ps1, wv_sb, 1, 0, 8)
                        v_chain(ps0, wv_sb, 0, 8, KC)
                        nc.vector.tensor_copy(V_sb[:, 0, 0:TQ], ps0)
                        v_chain(ps1, wv_sb, 1, 8, KC)
                        nc.vector.tensor_copy(V_sb[:, 1, 0:TQ], ps1)
                        trange = range(2, NTK)
                    else:
                        trange = range(NTK)
                    for t in trange:
                        ps = pvp.tile([128, TQ], F32, tag="pv")
                        v_chain(ps, wv_sb, t, 0, KC)
                        nc.vector.tensor_copy(
                            V_sb[:, t, vc * TQ:(vc + 1) * TQ], ps)

            # ---------------- per-head proj + attention -------------------------
            if True:
                wp_ctx = tc.tile_pool(name="wp", bufs=2)
                wp = wp_ctx.__enter__()
                wgp_ctx = tc.tile_pool(name="wgp", bufs=1)
                wgp = wgp_ctx.__enter__()
                qkp_ctx = tc.tile_pool(name="qkp", bufs=2)
                qkp = qkp_ctx.__enter__()
                pp_ctx = tc.tile_pool(name="pp", bufs=2, space="PSUM")
                pp = pp_ctx.__enter__()

                def load_qk(h):
                    wq_sb = wp.tile([128, KC, DH], F16, tag="wq", name=f"wq{h}")
                    wk_sb = wp.tile([128, KC, DH], F16, tag="wk", name=f"wk{h}")
                    nc.scalar.dma_start(out=wq_sb[:, 0:8, :], in_=wq[:, h, 0:8, :])
                    nc.sync.dma_start(out=wq_sb[:, 8:16, :], in_=wq[:, h, 8:16, :])
                    nc.scalar.dma_start(out=wk_sb[:, 0:8, :], in_=wk[:, h, 0:8, :])
                    nc.sync.dma_start(out=wk_sb[:, 8:16, :], in_=wk[:, h, 8:16, :])
                    return wq_sb, wk_sb

                def proj_head(h, out, qk, nextqk):
                    wq_sb, wk_sb = qk
                    # prefetch next head's q/k weights one head early
                    if h + 1 < HPC:
                        nextqk[h + 1] = load_qk(h + 1)
                    wg_sb = wgp.tile([128, KC, DH], F16, tag="wg", name=f"wg{h}")
                    nc.scalar.dma_start(out=wg_sb[:, 0:8, :], in_=wg[:, h, 0:8, :])
                    nc.sync.dma_start(out=wg_sb[:, 8:16, :], in_=wg[:, h, 8:16, :])
                    roped = {}
                    rkcol = rp.tile([128, NTK], F32, tag="rkcol", name=f"rkcol{h}")

                    # rms tail for a chunk, delayed one chunk so the PE never
                    # waits on the DVE-produced sq; the sum lands in row 0 of
                    # the chunk's own (already-drained) chain psum tile
                    def emit_rms(tname, c, sq, t1, rt, ps_prev):
                        cs = slice(c * TQ, (c + 1) * TQ)
                        ms = ps_prev[0:1, :]
                        nc.tensor.matmul(ms, ones_sb, sq, start=True, stop=True)
                        # rsqrt via exp(-0.5*ln(.)) -- keeps one act table
                        # set; rows 32/0 of one tile avoid in-place act
                        rn2 = sp.tile([33, TQ], F32, tag="rn2")
                        if tname == "q":
                            # rn_q = 1/sqrt(mean(q^2)+eps)
                            nc.scalar.activation(
                                rn2[32:33, :], ms, A.Ln,
                                bias=epsb_sb[:1, :], scale=1.0 / DH)
                        else:
                            # rn_k' = 1/sqrt(DH*(mean(k^2)+eps)) = rn_k/sqrt(DH)
                            nc.scalar.activation(
                                rn2[32:33, :], ms, A.Ln,
                                bias=eps128_sb[:1, :], scale=1.0)
                        nc.scalar.activation(
                            rn2[0:1, :], rn2[32:33, :], A.Exp, scale=-0.5)
                        if tname == "q":
                            rnb = sp.tile([128, TQ], F32, tag="arr")
                            nc.gpsimd.partition_broadcast(rnb, rn2[0:1, :])
                            nc.vector.tensor_mul(rt[:, cs], t1, rnb)
                        else:
                            # k-side rms scale rides the softmax-exp scale AP
                            nc.sync.dma_start(
                                out=rk_dram[h, cs], in_=rn2[0:1, :])
                            nc.sync.dma_start(
                                out=rkcol[:, c * 4:(c + 1) * 4],
                                in_=rk_dram[h, cs].rearrange(
                                    "(r j) -> j r", j=128),
                            )

                    pend = None
                    for tname, w_sb in (("q", wq_sb), ("k", wk_sb)):
                        rt = rp.tile([128, L], F16, tag=f"{tname}r", name=f"{tname}r{h}")
                        for c in range(NTQ):
                            cs = slice(c * TQ, (c + 1) * TQ)
                            ps = pp.tile([128, TQ], F32, tag="mm")
                            for kc in range(KC):
                                nc.tensor.matmul(
                                    ps,
                                    w_sb[:, kc, :],
                                    xT_sb[:, 4 * c:4 * c + 4, kc, :],
                                    start=(kc == 0),
                                    stop=(kc == KC - 1),
                                )
                            qt = qkp.tile([128, TQ], F16, tag="qt")
                            nc.vector.tensor_copy(qt, ps)
                            qsw = qkp.tile([128, TQ], F16, tag="qsw")
                            nc.sync.dma_start(out=qsw[0:64, :], in_=qt[64:128, :])
                            nc.sync.dma_start(out=qsw[64:128, :], in_=qt[0:64, :])
                            sq = sp.tile([128, TQ], F16, tag="sq")
                            nc.vector.tensor_mul(sq, qt, qt)
                            # rope halves: t1 = qt*cos ; qsw *= sin_signed
                            t1 = sp.tile([128, TQ], F16, tag="t1")
                            nc.vector.tensor_mul(t1, qt, cos_sb[:, cs])
                            nc.vector.tensor_mul(qsw, qsw, sin_sb[:, cs])
                            if tname == "q":
                                nc.vector.tensor_add(t1, t1, qsw)
                            else:
                                # k rope lands in rt; rms scale applied at exp
                                nc.vector.tensor_add(rt[:, cs], t1, qsw)
                            if pend is not None:
                                emit_rms(*pend)
                            pend = (tname, c, sq, t1, rt, ps)
                            yield
                        roped[tname] = rt
                    # gate: store exp(-g); sigmoid finished in the attn epilogue
                    gt = gp.tile([128, L], F16, tag="gt", name=f"g{h}")
                    for c in range(NTQ):
                        cs = slice(c * TQ, (c + 1) * TQ)
                        ps = pp.tile([128, TQ], F32, tag="mm")
                        for kc in range(KC):
                            nc.tensor.matmul(
                                ps,
                                wg_sb[:, kc, :],
                                xT_sb[:, 4 * c:4 * c + 4, kc, :],
                                start=(kc == 0),
                                stop=(kc == KC - 1),
                            )
                        nc.scalar.activation(gt[:, cs], ps, A.Exp, scale=-1.0)
                        if pend is not None:
                            emit_rms(*pend)
                            pend = None
                        yield
                    out.update(q=roped["q"], k=roped["k"], gt=gt, rkcol=rkcol)

                def attn_head(h, qr, kr, gt, rkcol, corder=None):
                    for c in (corder if corder is not None else range(NTQ)):
                        cs = slice(c * TQ, (c + 1) * TQ)
                        nkt = 4 * c + 4
                        pavt = pav.tile([128, TQ], F32, tag="av")
                        esum = sp.tile([128, TQ], F16, tag="esum", name=f"es{h}_{c}")
                        pend = []
                        last_pst = None
                        for kt in range(nkt):
                            r = kt - 4 * c
                            co = max(0, 128 * r)   # masked-out column prefix
                            ncs = slice(c * TQ + co, (c + 1) * TQ)
                            pst = pss.tile([128, TQ], F32, tag="s")
                            last_pst = pst
                            nc.tensor.matmul(
                                pst[:, co:],
                                kr[:, kt * 128:(kt + 1) * 128],
                                qr[:, ncs],
                                start=True,
                                stop=True,
                            )
                            e = epool.tile([128, TQ], F16, tag="e")
                            nc.scalar.activation(
                                e[:, co:], pst[:, co:], A.Exp,
                                scale=rkcol[:, kt:kt + 1], bias=ebias_sb[:, :],
                            )
                            if r >= 0:
                                nc.vector.tensor_mul(
                                    e[:, co:], e[:, co:], mask_sb[:, 0:TQ - co])
                            if kt == 0:
                                nc.vector.tensor_copy(esum, e)
                            else:
                                nc.vector.tensor_add(
                                    esum[:, co:], esum[:, co:], e[:, co:])
                            pend.append((kt, e, co))
                            if len(pend) > 2:
                                pkt, pe, pco = pend.pop(0)
                                nc.tensor.matmul(
                                    pavt[:, pco:],
                                    V_sb[:, pkt, h * DH:(h + 1) * DH],
                                    pe[:, pco:],
                                    start=(pkt == 0), stop=False,
                                )
                            if kt % 2 == 1:
                                yield
                        while pend:
                            pkt, pe, pco = pend.pop(0)
                            nc.tensor.matmul(
                                pavt[:, pco:], V_sb[:, pkt, h * DH:(h + 1) * DH],
                                pe[:, pco:],
                                start=(pkt == 0), stop=(len(pend) == 0),
                            )
                        # denominator sum lands in row 0 of the chunk's last
                        # (already-consumed) score psum tile
                        den = last_pst[0:1, :]
                        nc.tensor.matmul(den, ones_sb, esum, start=True, stop=True)
                        rd = sp.tile([33, TQ], F32, tag="rn2", name=f"rd{h}_{c}")
                        nc.vector.reciprocal_approx_fast(out=rd[0:1, :], in_=den)
                        rdb = sp.tile([128, TQ], F32, tag="arr", name=f"rdb{h}_{c}")
                        nc.gpsimd.partition_broadcast(rdb, rd[0:1, :])
                        # finish the sigmoid: gate = 1/(1+exp(-g))
                        gden = sp.tile([128, TQ], F32, tag="arr", name=f"gd{h}_{c}")
                        nc.vector.tensor_scalar_add(gden, gt[:, cs], 1.0)
                        grec = gwp.tile([128, TQ], F32, tag="grec", name=f"gr{h}_{c}")
                        nc.vector.reciprocal_approx_fast(out=grec, in_=gden)
                        tn = sp.tile([128, TQ], F16, tag="tn")
                        # tn = av / den
                        nc.vector.scalar_tensor_tensor(
                            out=tn, in0=pavt, scalar=1.0, in1=rdb,
                            op0=mybir.AluOpType.mult, op1=mybir.AluOpType.mult)
                        nc.vector.tensor_mul(gated[:, h, cs], tn, grec)
                        yield ("c", c)

                def zip2(pg, ag, ratio=3):
                    done_p = done_a = False
                    while not (done_p and done_a):
                        if not done_p:
                            try:
                                next(pg)
                            except StopIteration:
                                done_p = True
                        for _ in range(ratio):
                            if done_a:
                                break
                            try:
                                next(ag)
                            except StopIteration:
                                done_a = True

                ag = None
                outs = {}
                nextqk = {0: load_qk(0)}
                for h in range(HPC):
                    out = {}
                    pg = proj_head(h, out, nextqk.pop(h), nextqk)
                    if ag is None:
                        for _ in pg:
                            pass
                    else:
                        zip2(pg, ag)
                    outs[h] = out
                    # the last head's attention runs longest-chunk-first so
                    # the final out-proj rows only wait on the short chunk
                    corder = [3, 2, 1, 0] if h == HPC - 1 else None
                    ag = attn_head(h, out["q"], out["k"], out["gt"],
                                   out["rkcol"], corder)

                # free proj-phase pools; xT no longer needed after proj7
                pp_ctx.__exit__(None, None, None)
                qkp_ctx.__exit__(None, None, None)
                wgp_ctx.__exit__(None, None, None)
                wp_ctx.__exit__(None, None, None)
                xtp.__exit__(None, None, None)

                # ---------------- out projection, zipped with attn(7) ------------
                with (
                    tc.tile_pool(name="wo", bufs=1) as wo,
                    tc.tile_pool(name="yp", bufs=2) as yp,
                    tc.tile_pool(name="py", bufs=2, space="PSUM") as pyp,
                ):
                    wout_sb = wo.tile([128, HPC, HID], F16, tag="wout")
                    for hc in range(HPC):
                        eng = dma_engs[hc % 3]
                        eng.dma_start(out=wout_sb[:, hc, :], in_=wout[:, hc, :])

                    def outproj_chunk(trange):
                        for t in trange:
                            ysb = yp.tile([128, HID], F16, tag="y")
                            for oc in range(NTQ):
                                ocs = slice(oc * TQ, (oc + 1) * TQ)
                                ps = pyp.tile([128, TQ], F32, tag="ym")
                                for hc in range(HPC):
                                    nc.tensor.matmul(
                                        ps,
                                        gated[:, hc, t * 128:(t + 1) * 128],
                                        wout_sb[:, hc, ocs],
                                        start=(hc == 0),
                                        stop=(hc == HPC - 1),
                                    )
                                if oc % 2 == 0:
                                    nc.vector.tensor_copy(ysb[:, ocs], ps)
                                else:
                                    nc.scalar.copy(ysb[:, ocs], ps)
                                # stream each quarter out as soon as it is copied
                                eng = nc.sync if oc % 2 == 0 else nc.scalar
                                eng.dma_start(
                                    out=y[t * 128:(t + 1) * 128, ocs],
                                    in_=ysb[:, ocs])

                    # interleave: after attn(7) finishes chunk c, emit the
                    # out-proj rows that depend on it (t = 4c .. 4c+3)
                    for step in ag:
                        if isinstance(step, tuple) and step[0] == "c":
                            c = step[1]
                            outproj_chunk(range(4 * c, 4 * c + 4))

                for ctx in (pav_ctx, pss_ctx, ep_ctx, gw_ctx,
                            sp_ctx, gp_ctx, rp_ctx):
                    ctx.__exit__(None, None, None)

    nc.compile()
    return nc


def _host_tables():
    half = DH // 2
    inv_freq = 1.0 / (THETA ** (np.arange(half, dtype=np.float64) * 2.0 / DH))
    pos = np.arange(L, dtype=np.float64)
    ang = pos[:, None] * inv_freq[None, :]          # (L, 64)
    cos = np.cos(ang).T                             # (64, L)
    sin = np.sin(ang).T
    cosT = np.concatenate([cos, cos], axis=0).astype(np.float16)        # (128, L)
    sinT = np.concatenate([-sin, sin], axis=0).astype(np.float16)
    j = np.arange(128)[:, None]
    i = np.arange(TQ)[None, :]
    mask = (j <= i).astype(np.float16)                                  # (128,512)
    return cosT, sinT, mask


def _tile_x(xb):
    # (HID, L) -> [k, tch, kc, tin] = [128, NTK, KC, 128]
    return np.ascontiguousarray(
        xb.reshape(KC, 128, NTK, 128).transpose(1, 2, 0, 3)).astype(np.float16)


def _tile_w(w, inner, last):
    # (HID, inner*last) -> [k, inner, kc, last]
    return np.ascontiguousarray(
        w.reshape(KC, 128, inner, last).transpose(1, 2, 0, 3)).astype(np.float16)


def _run(hidden_states, W_qkvg, W_out, trace=False, trace_cores=None):
    key = "nc"
    if key not in _NC_CACHE:
        _install_act_tables()
        _NC_CACHE[key] = build_nc()
    nc = _NC_CACHE[key]

    hidden_states = np.asarray(hidden_states)
    W_qkvg = np.asarray(W_qkvg)
    W_out = np.asarray(W_out)

    cosT, sinT, mask = _host_tables()
    QKV = 3 * H * DH

    in_maps = []
    for core in range(8):
        b, g = divmod(core, 2)
        cols = slice(g * HPC * DH, (g + 1) * HPC * DH)
        wv_cols = W_qkvg[:, 2 * H * DH:3 * H * DH][:, cols]
        in_maps.append({
            "xT": _tile_x(np.ascontiguousarray(hidden_states[b].T)),
            "wq": _tile_w(W_qkvg[:, 0 * H * DH:1 * H * DH][:, cols], HPC, DH),
            "wk": _tile_w(W_qkvg[:, 1 * H * DH:2 * H * DH][:, cols], HPC, DH),
            "wv": _tile_w(wv_cols, 2, TQ),
            "wg": _tile_w(W_qkvg[:, QKV:][:, cols], HPC, DH),
            "wout": np.ascontiguousarray(
                W_out[cols, :].reshape(HPC, 128, HID).transpose(1, 0, 2)
            ).astype(np.float16),
            "cosT": cosT,
            "sinT": sinT,
            "masks": mask,
        })

    kw = {}
    if trace:
        kw["trace"] = True
        if trace_cores is not None:
            kw["trace_cores"] = trace_cores
    res = run_bass_kernel_spmd(nc, in_maps, core_ids=list(range(8)), **kw)

    out = np.empty((B, L, HID), dtype=np.float32)
    for b in range(B):
        out[b] = (res.results[2 * b]["y"].astype(np.float32)
                  + res.results[2 * b + 1]["y"].astype(np.float32))
    return out, res


def kernel(hidden_states, W_qkvg, W_out):
    trace = os.environ.get("KERNEL_TRACE", "0") == "1"
    out, res = _run(hidden_states, W_qkvg, W_out, trace=trace)
    kernel.last_results = res
    return out


if __name__ == "__main__":
    rng = np.random.default_rng(0)
    hs = rng.standard_normal((B, L, HID), dtype=np.float32)
    wqkvg = (rng.standard_normal((HID, QKV_ := 3 * H * DH + HID), dtype=np.float32) * 0.02)
    wout = (rng.standard_normal((HID, HID), dtype=np.float32) * 0.02)
    out = kernel(hs, wqkvg, wout)
    print(out.shape, out.dtype)
